# revision 1
# baseline (speedup 1.0000x reference)
"""Coupled-attention module as a distributed Bass/Tile kernel on 8 TRN2 cores.

Math notes (exact algebra, not approximations):
- The differential-attention scores are constant along the softmax axis, so
  softmax yields exactly uniform 1/S weights: diff_vector collapses to the
  per-batch mean of (y @ dv_w + dv_b), broadcast over sequence. dq/dk are dead.
- Sharding: rows of the flattened (B*S, H) activations, 256 per core; cores
  0-3 own batch 0, 4-7 batch 1. Each core redundantly computes full-batch K/V
  (cheaper than any reshard collective at this scale).
- All activations live channel-major [C, rows] on chip, so weights feed the
  PE as natural [K, M] lhsT tiles, and the two sequence-axis softmaxes in the
  gating network reduce along the free dim. Their denominators are summed
  across the 4-core batch group with tiny AllReduces.
- Compute in bf16 with fp32 accumulation (all GEMMs), exp/tanh/sigmoid on ACT.
- The AllReduce-independent halves of the v_gamma and van_out GEMMs are
  pre-accumulated into SBUF while the collectives are in flight, keeping the
  PE busy (and its HAM clock warm) through the bubbles.
"""

import numpy as np
import ml_dtypes

import concourse.bass as bass
import concourse.mybir as mybir
import concourse.tile as tile
from concourse import bacc
from concourse.bass_utils import run_bass_kernel_spmd

B, S, H = 2, 1024, 768
NH, DH = 12, 64
P = 128
RV = 256            # rows per core
KC = H // P         # 6 channel chunks
JC = S // P         # 8 sequence chunks
GROUPS = [[0, 1, 2, 3], [4, 5, 6, 7]]
SCALE = 1.0 / 8.0   # 1/sqrt(DH)

bf16 = mybir.dt.bfloat16
f32 = mybir.dt.float32
AF = mybir.ActivationFunctionType
ALU = mybir.AluOpType
nbf16 = ml_dtypes.bfloat16

W768 = ["vq_w", "vk_w", "vv_w", "dv_w", "WD_w", "van_fc_w", "WV_w", "diff_fc_w",
        "diff_fus_w", "van_fus_w", "nf_w", "final_w"]
W1536 = ["d_theta_w", "v_gamma_w", "diff_out_w", "van_out_w"]
BIAS = ["vq_b", "vk_b", "dv_b", "van_fc_b", "d_theta_b", "diff_fc_b",
        "v_gamma_b", "diff_out_b", "van_out_b", "diff_fus_b", "van_fus_b",
        "nf_b", "final_b"]


def build(has_vvb: bool):
    nc = bacc.Bacc(None, target_bir_lowering=False, debug=False, num_devices=8)

    xT_d = nc.dram_tensor("xT", [H, RV], bf16, kind="ExternalInput")
    yT_d = nc.dram_tensor("yT", [H, S], bf16, kind="ExternalInput")
    wd = {}
    for w in W768:
        wd[w] = nc.dram_tensor(w, [H, H], bf16, kind="ExternalInput")
    for w in W1536:
        wd[w] = nc.dram_tensor(w, [2 * H, H], bf16, kind="ExternalInput")
    wd["gate_w"] = nc.dram_tensor("gate_w", [2 * H, 1], bf16, kind="ExternalInput")
    wd["nf_out_w"] = nc.dram_tensor("nf_out_w", [2 * H, 1], bf16, kind="ExternalInput")
    bd = {}
    for b in BIAS:
        bd[b] = nc.dram_tensor(b, [H], f32, kind="ExternalInput")
    if has_vvb:
        bd["vv_b"] = nc.dram_tensor("vv_b", [H], f32, kind="ExternalInput")
    out_d = nc.dram_tensor("outT", [H, RV], f32, kind="ExternalOutput")

    with tile.TileContext(nc, num_cores=8) as tc:
        with (
            tc.tile_pool(name="wpool", bufs=5) as wp,
            tc.tile_pool(name="wsmall", bufs=2) as wsp,
            tc.tile_pool(name="acts", bufs=1) as ap,
            tc.tile_pool(name="loop", bufs=2) as lp,
            tc.tile_pool(name="psum", bufs=8, space="PSUM") as pp,
            tc.tile_pool(name="dram", bufs=4, space="DRAM") as dp,
        ):
            def wtile(name, half=None):
                t = wp.tile([P, KC, H], bf16, name=f"w_{name}_{half}", tag="w")
                src = wd[name]
                if half is not None:
                    src = src[half * H:(half + 1) * H, :]
                src = src.rearrange("(kc p) n -> kc p n", p=P)
                for kc in range(KC):
                    nc.sync.dma_start(t[:, kc, :], src[kc])
                return t

            def btile(name):
                t = ap.tile([P, KC], f32, name=f"b_{name}")
                nc.sync.dma_start(t[:], bd[name].rearrange("(c p) -> p c", p=P))
                return t

            # ---------------- Q projection first: minimal-dependency PE work
            b_vq = btile("vq_b")
            xT = ap.tile([P, KC, RV], bf16, name="xT")
            for kc in range(KC):
                nc.sync.dma_start(xT[:, kc, :], xT_d.rearrange(
                    "(kc p) n -> kc p n", p=P)[kc])
            w_vq = wtile("vq_w")
            qT = ap.tile([P, KC, RV], bf16, name="qT")
            for mc in range(KC):
                ps = pp.tile([P, RV], f32, name=f"qps{mc}", tag="sps", bufs=3)
                for kc in range(KC):
                    nc.tensor.matmul(ps[:], w_vq[:, kc, mc * P:(mc + 1) * P],
                                     xT[:, kc, :],
                                     start=(kc == 0), stop=(kc == KC - 1))
                nc.scalar.activation(qT[:, mc, :], ps[:], AF.Identity,
                                     bias=b_vq[:, mc:mc + 1])

            b_vk = btile("vk_b")
            b_dv = btile("dv_b")
            yT = ap.tile([P, KC, S], bf16, name="yT")
            for kc in range(KC):
                nc.sync.dma_start(yT[:, kc, :], yT_d.rearrange(
                    "(kc p) n -> kc p n", p=P)[kc])

            ones64 = ap.tile([1, 64], f32, name="ones64")
            nc.vector.memset(ones64[:], 1.0)
            ones128 = ap.tile([1, P], f32, name="ones128")
            nc.vector.memset(ones128[:], 1.0)

            # ---------------- K projection (full batch, channel-major) ------
            w_vk = wtile("vk_w")
            kT = ap.tile([P, KC, S], bf16, name="kT")
            for mc in range(KC):
                for nh in range(2):
                    ps = pp.tile([P, 512], f32, name=f"kps{mc}_{nh}", tag="big", bufs=3)
                    for kc in range(KC):
                        nc.tensor.matmul(
                            ps[:], w_vk[:, kc, mc * P:(mc + 1) * P],
                            yT[:, kc, nh * 512:(nh + 1) * 512],
                            start=(kc == 0), stop=(kc == KC - 1))
                    nc.scalar.activation(kT[:, mc, nh * 512:(nh + 1) * 512], ps[:],
                                         AF.Identity, bias=b_vk[:, mc:mc + 1])

            # ---------------- V projection (row-major + ones col) -----------
            w_vv = wtile("vv_w")
            v_aug = ap.tile([P, JC, NH, DH + 1], bf16, name="v_aug")
            nc.vector.memset(v_aug[:, :, :, DH:DH + 1], 1.0)
            for jc in range(JC):
                for cg in range(2):
                    ps = pp.tile([P, 384], f32, name=f"vps{jc}_{cg}", tag="big", bufs=3)
                    for kc in range(KC):
                        nc.tensor.matmul(
                            ps[:], yT[:, kc, jc * P:(jc + 1) * P],
                            w_vv[:, kc, cg * 384:(cg + 1) * 384],
                            start=(kc == 0), stop=(kc == KC - 1))
                    nc.vector.tensor_copy(
                        v_aug[:, jc, cg * 6:(cg + 1) * 6, 0:DH],
                        ps[:].rearrange("p (h d) -> p h d", d=DH))

            # ---------------- diff-branch constants (per batch) -------------
            # m = mean_s(y) @ dv_w + dv_b ; theta1 = tanh(m @ WD_w)
            # bias1 = theta1 @ d_theta_w[:H] + d_theta_b
            # bias2 = m @ diff_out_w[:H] + diff_out_b
            yb = ap.tile([P, KC], f32, name="yb")
            ybt = ap.tile([P, KC], bf16, name="ybt")
            for kc in range(KC):
                nc.vector.tensor_reduce(yb[:, kc:kc + 1], yT[:, kc, :],
                                        axis=mybir.AxisListType.X, op=ALU.add)
            nc.vector.tensor_scalar_mul(ybt[:], yb[:], 1.0 / S)

            def vec_chain(w_t, rhs_t, func, bias_t, out_dt, name):
                out = ap.tile([P, KC], out_dt, name=name)
                for mc in range(KC):
                    ps = pp.tile([P, 1], f32, name=f"{name}ps{mc}", tag="sps", bufs=3)
                    for kc in range(KC):
                        nc.tensor.matmul(ps[:], w_t[:, kc, mc * P:(mc + 1) * P],
                                         rhs_t[:, kc:kc + 1],
                                         start=(kc == 0), stop=(kc == KC - 1))
                    nc.scalar.activation(out[:, mc:mc + 1], ps[:], func,
                                         bias=(bias_t[:, mc:mc + 1]
                                               if bias_t is not None else 0.0))
                return out

            w_dv = wtile("dv_w")
            m32 = vec_chain(w_dv, ybt, AF.Identity, b_dv, f32, "m32")
            mbf = ap.tile([P, KC], bf16, name="mbf")
            nc.vector.tensor_copy(mbf[:], m32[:])
            w_WD = wtile("WD_w")
            th1 = vec_chain(w_WD, mbf, AF.Tanh, None, bf16, "th1")
            w_dth0 = wtile("d_theta_w", half=0)
            b_dth = btile("d_theta_b")
            bias1 = vec_chain(w_dth0, th1, AF.Identity, b_dth, f32, "bias1")
            w_dout0 = wtile("diff_out_w", half=0)
            b_dout = btile("diff_out_b")
            bias2 = vec_chain(w_dout0, mbf, AF.Identity, b_dout, f32, "bias2")

            # ---------------- attention (12 heads, 256 own queries) ---------
            if has_vvb:
                b_vv = btile("vv_b")
            vanT = ap.tile([P, KC, RV], bf16, name="vanT")

            def head_tail(h, pv):
                hc, ho = h // 2, (h % 2) * 64
                invZ = lp.tile([1, RV], f32, name=f"invZ{h}", tag="invZ")
                nc.vector.reciprocal(invZ[:], pv[DH:DH + 1, :])
                bc = pp.tile([64, RV], f32, name=f"bc{h}", tag="sps", bufs=3)
                nc.tensor.matmul(bc[:], ones64[:], invZ[:], start=True, stop=True)
                bcs = lp.tile([64, RV], f32, name=f"bcs{h}", tag="bcs")
                nc.vector.tensor_copy(bcs[:], bc[:])
                nc.vector.tensor_mul(vanT[ho:ho + 64, hc, :], pv[0:DH, :], bcs[:])
                if has_vvb:
                    nc.vector.tensor_scalar_add(vanT[ho:ho + 64, hc, :],
                                                vanT[ho:ho + 64, hc, :],
                                                b_vv[ho:ho + 64, hc:hc + 1])

            for hp in range(NH // 2):
                h0, h1 = 2 * hp, 2 * hp + 1
                hc = hp
                e0 = lp.tile([P, JC, RV], bf16, name=f"expT{h0}", tag="expT", bufs=4)
                e1_ = lp.tile([P, JC, RV], bf16, name=f"expT{h1}", tag="expT", bufs=4)
                pv0 = pp.tile([DH + 1, RV], f32, name=f"pv{h0}", tag="pv", bufs=2)
                pv1 = pp.tile([DH + 1, RV], f32, name=f"pv{h1}", tag="pv", bufs=2)
                for jc in range(JC):
                    for (h, ex) in ((h0, e0), (h1, e1_)):
                        ho = (h % 2) * 64
                        sps = pp.tile([P, RV], f32, name=f"sps{h}_{jc}",
                                      tag="sps", bufs=3)
                        nc.tensor.matmul(sps[:],
                                         kT[ho:ho + 64, hc, jc * P:(jc + 1) * P],
                                         qT[ho:ho + 64, hc, :],
                                         start=True, stop=True)
                        nc.scalar.activation(ex[:, jc, :], sps[:], AF.Exp,
                                             scale=SCALE)
                for jc in range(JC):
                    nc.tensor.matmul(pv0[:], v_aug[:, jc, h0, :], e0[:, jc, :],
                                     start=(jc == 0), stop=(jc == JC - 1))
                    nc.tensor.matmul(pv1[:], v_aug[:, jc, h1, :], e1_[:, jc, :],
                                     start=(jc == 0), stop=(jc == JC - 1))
                head_tail(h0, pv0)
                head_tail(h1, pv1)

            # ---------------- gating network ---------------------------------
            def gemm(pairs, func, bias_t=None, accum_t=None, name="g",
                     out_dt=bf16, pre=None):
                out = ap.tile([P, KC, RV], out_dt, name=name)
                nmm = len(pairs) * KC
                for mc in range(KC):
                    ps = pp.tile([P, RV], f32, name=f"{name}ps{mc}", tag="big", bufs=3)
                    i = 0
                    for wt, at in pairs:
                        for kc in range(KC):
                            nc.tensor.matmul(ps[:],
                                             wt[:, kc, mc * P:(mc + 1) * P],
                                             at[:, kc, :],
                                             start=(i == 0), stop=(i == nmm - 1))
                            i += 1
                    src = ps
                    if pre is not None:
                        tmp = lp.tile([P, RV], f32, name=f"{name}pre{mc}",
                                      tag="pretmp")
                        nc.vector.tensor_add(tmp[:], ps[:], pre[:, mc, :])
                        src = tmp
                    nc.scalar.activation(
                        out[:, mc, :], src[:], func,
                        bias=(bias_t[:, mc:mc + 1] if bias_t is not None else 0.0),
                        accum_out=(accum_t[:, mc:mc + 1]
                                   if accum_t is not None else None))
                return out

            def allreduce6(part, name):
                ci = dp.tile([P, KC], f32, name=f"ci_{name}")
                co = dp.tile([P, KC], f32, name=f"co_{name}")
                nc.sync.dma_start(ci[:], part[:])
                nc.gpsimd.collective_compute(
                    "AllReduce", ALU.add, replica_groups=GROUPS,
                    ins=[ci[:]], outs=[co[:]])
                z = ap.tile([P, KC], f32, name=f"z_{name}")
                nc.sync.dma_start(z[:], co[:])
                return z

            w_vfc = wtile("van_fc_w")
            b_vfc = btile("van_fc_b")
            theta2 = gemm([(w_vfc, vanT)], AF.Tanh, bias_t=b_vfc, name="theta2")

            w_dth1 = wtile("d_theta_w", half=1)
            part1 = ap.tile([P, KC], f32, name="part1")
            e1 = gemm([(w_dth1, theta2)], AF.Exp, bias_t=bias1, accum_t=part1,
                      name="e1")
            z1 = allreduce6(part1, "z1")

            # --- AllReduce-1 bubble fillers (independent of z1) -------------
            w_WV = wtile("WV_w")
            gamma1 = gemm([(w_WV, vanT)], AF.Tanh, name="gamma1")
            w_vg0 = wtile("v_gamma_w", half=0)
            b_vg = btile("v_gamma_b")
            z2a = gemm([(w_vg0, gamma1)], AF.Identity, bias_t=b_vg, name="z2a",
                       out_dt=f32)
            w_vo0 = wtile("van_out_w", half=0)
            b_vo = btile("van_out_b")
            voa = gemm([(w_vo0, vanT)], AF.Identity, bias_t=b_vo, name="voa",
                       out_dt=f32)

            s1 = ap.tile([P, KC], f32, name="s1")
            nc.vector.reciprocal(s1[:], z1[:])
            nc.vector.tensor_mul(s1[:], s1[:], m32[:])
            dth = ap.tile([P, KC, RV], bf16, name="dth")
            for mc in range(KC):
                nc.vector.tensor_scalar_mul(dth[:, mc, :], e1[:, mc, :],
                                            s1[:, mc:mc + 1])

            w_dfc = wtile("diff_fc_w")
            b_dfc = btile("diff_fc_b")
            gamma2 = gemm([(w_dfc, dth)], AF.Tanh, bias_t=b_dfc, name="gamma2")

            w_vg1 = wtile("v_gamma_w", half=1)
            part2 = ap.tile([P, KC], f32, name="part2")
            e2 = gemm([(w_vg1, gamma2)], AF.Exp, accum_t=part2, pre=z2a,
                      name="e2")
            z2 = allreduce6(part2, "z2")

            # --- AllReduce-2 bubble fillers (diff branch tail) --------------
            w_dout1 = wtile("diff_out_w", half=1)
            dout = gemm([(w_dout1, dth)], AF.Tanh, bias_t=bias2, name="dout")
            w_dfus = wtile("diff_fus_w")
            b_dfus = btile("diff_fus_b")
            dfus = gemm([(w_dfus, dout)], AF.Tanh, bias_t=b_dfus, name="dfus")

            s2 = ap.tile([P, KC], f32, name="s2")
            nc.vector.reciprocal(s2[:], z2[:])
            ag = ap.tile([P, KC, RV], bf16, name="ag")
            for mc in range(KC):
                nc.vector.scalar_tensor_tensor(
                    ag[:, mc, :], e2[:, mc, :], s2[:, mc:mc + 1],
                    vanT[:, mc, :], op0=ALU.mult, op1=ALU.mult)

            w_vo1 = wtile("van_out_w", half=1)
            vout = gemm([(w_vo1, ag)], AF.Tanh, pre=voa, name="vout")
            w_vfus = wtile("van_fus_w")
            b_vfus = btile("van_fus_b")
            vfus = gemm([(w_vfus, vout)], AF.Tanh, bias_t=b_vfus, name="vfus")

            # gate (M=1 GEMM over both fusion tensors)
            def vec_unit(wname, act_pairs, name):
                wt = wsp.tile([P, 2 * KC, 1], bf16, name=f"ws_{name}", tag="ws")
                nc.sync.dma_start(wt[:], wd[wname].rearrange(
                    "(c p) o -> p c o", p=P))
                ps = pp.tile([1, RV], f32, name=f"{name}ps", tag="sps", bufs=3)
                i = 0
                for at, base in act_pairs:
                    for kc in range(KC):
                        nc.tensor.matmul(ps[:], wt[:, base + kc, :],
                                         at[:, kc, :],
                                         start=(i == 0), stop=(i == 2 * KC - 1))
                        i += 1
                out = ap.tile([1, RV], f32, name=f"v_{name}")
                nc.scalar.activation(out[:], ps[:], AF.Sigmoid)
                return out

            g = vec_unit("gate_w", [(dfus, 0), (vfus, KC)], "gate")
            gbc = pp.tile([P, RV], f32, name="gbc", tag="sps", bufs=3)
            nc.tensor.matmul(gbc[:], ones128[:], g[:], start=True, stop=True)

            fus = ap.tile([P, KC, RV], bf16, name="fus")
            for mc in range(KC):
                t1 = lp.tile([P, RV], bf16, name=f"ft1_{mc}", tag="ft1")
                nc.vector.tensor_sub(t1[:], vfus[:, mc, :], dfus[:, mc, :])
                t2 = lp.tile([P, RV], bf16, name=f"ft2_{mc}", tag="ft2")
                nc.vector.tensor_mul(t2[:], t1[:], gbc[:])
                nc.vector.tensor_add(fus[:, mc, :], t2[:], dfus[:, mc, :])

            w_nf = wtile("nf_w")
            b_nf = btile("nf_b")
            tnf = gemm([(w_nf, fus)], AF.Identity, bias_t=b_nf, name="tnf")
            nfv = vec_unit("nf_out_w", [(vanT, 0), (tnf, KC)], "nf")
            nbc = pp.tile([P, RV], f32, name="nbc", tag="sps", bufs=3)
            nc.tensor.matmul(nbc[:], ones128[:], nfv[:], start=True, stop=True)

            w_fin = wtile("final_w")
            b_fin = btile("final_b")
            ft = gemm([(w_fin, fus)], AF.Tanh, bias_t=b_fin, name="ftanh")
            outT = ap.tile([P, KC, RV], f32, name="outT")
            for mc in range(KC):
                nc.vector.tensor_mul(outT[:, mc, :], ft[:, mc, :], nbc[:])
            nc.sync.dma_start(out_d.rearrange("(mc p) n -> p mc n", p=P), outT[:])

    nc.compile()
    return nc


_CACHE = {}


def kernel(**inputs):
    x = np.asarray(inputs["x"], np.float32)
    y = np.asarray(inputs["y"], np.float32)
    has_vvb = bool(np.any(np.asarray(inputs["vv_b"]) != 0))
    if has_vvb not in _CACHE:
        _CACHE[has_vvb] = build(has_vvb)
    nc = _CACHE[has_vvb]

    xt = np.ascontiguousarray(x.reshape(B * S, H).T).astype(nbf16)   # [H, 2048]
    yts = [np.ascontiguousarray(y[b].T).astype(nbf16) for b in range(B)]

    base = {}
    for w in W768 + W1536 + ["gate_w", "nf_out_w"]:
        base[w] = np.asarray(inputs[w], np.float32).astype(nbf16)
    for b in BIAS:
        base[b] = np.ascontiguousarray(np.asarray(inputs[b], np.float32))
    if has_vvb:
        base["vv_b"] = np.ascontiguousarray(np.asarray(inputs["vv_b"], np.float32))

    in_maps = []
    for c in range(8):
        bat = c // 4
        m = dict(base)
        m["xT"] = np.ascontiguousarray(xt[:, c * RV:(c + 1) * RV])
        m["yT"] = yts[bat]
        in_maps.append(m)

    res = run_bass_kernel_spmd(nc, in_maps, core_ids=list(range(8)))
    full = np.concatenate([res.results[c]["outT"] for c in range(8)], axis=1)
    return np.ascontiguousarray(full.T.reshape(B, S, H)).astype(np.float32)


if __name__ == "__main__":
    rng = np.random.default_rng(0)
    ins = {"x": rng.standard_normal((B, S, H)).astype(np.float32),
           "y": rng.standard_normal((B, S, H)).astype(np.float32)}
    for w in W768 + W1536:
        shp = (H, H) if w in W768 else (2 * H, H)
        ins[w] = (rng.standard_normal(shp) * 0.02).astype(np.float32)
    ins["gate_w"] = (rng.standard_normal((2 * H, 1)) * 0.02).astype(np.float32)
    ins["nf_out_w"] = (rng.standard_normal((2 * H, 1)) * 0.02).astype(np.float32)
    for b in BIAS + ["vv_b"]:
        ins[b] = np.zeros(H, np.float32)
    out = kernel(**ins)
    print("out", out.shape, out.dtype, np.abs(out).mean())



# revision 15
# speedup vs baseline: 1.1539x; 1.1539x over previous
"""Coupled-attention module as a distributed Bass/Tile kernel on 8 TRN2 cores.

Key design points (numerics validated vs the reference in numpy, ~5.5e-3):
- Differential-attention scores are constant along the softmax axis ->
  diff_vector collapses to the per-batch mean of (y @ dv_w + dv_b); dq/dk dead.
- bias1 = th1 @ d_theta_w[:H] + d_theta_b shifts e1 columns uniformly along
  the sequence-softmax axis, so it cancels EXACTLY in d_theta: th1, WD_w,
  d_theta_w[:H], d_theta_b are dropped. Same cancellation kills v_gamma_b.
- Noise-insensitive gating GEMMs (e1, z2a, gamma2, e2, dout, vout, tnf) run
  as fp8e4m3 DoubleRow with weights at 16x: K=256 per instruction, which
  HALVES instruction count (measured: a DR matmul costs the same as a bf16
  matmul of equal output width). Everything touching the output trunk
  (q/k/v, theta2/gamma1/voa from vanT, dfus/vfus/final) stays bf16.
- Sigmoids via exp + reciprocal so one ACT table (exp/tanh/identity/copy)
  serves the whole kernel - no mid-kernel table loads.
- Sharding: rows of (B*S, H), 256/core; cores 0-3 batch 0, 4-7 batch 1.
  Full-batch K/V computed redundantly per core (cheaper than a reshard).
  Two 3KB AllReduces for the sequence-softmax denominators, preceded by a
  warmup collective; independent GEMMs fill the collective bubbles.
- K-projection chunks are interleaved with attention head-pairs (chunk hp
  feeds exactly head pair hp), so attention starts early.
- Weight DMAs are split in half across the SP and ACT hardware DGE queues
  (measured per-core DMA ~210 GB/s; total input traffic ~21 MB).
"""

import numpy as np
import ml_dtypes

import concourse.bass as bass
import concourse.mybir as mybir
import concourse.tile as tile
from concourse import bacc
from concourse.bass_utils import run_bass_kernel_spmd

B, S, H = 2, 1024, 768
NH, DH = 12, 64
P = 128
RV = 256            # rows per core
KC = H // P         # 6 channel chunks
KP = KC // 2        # 3 DoubleRow chunk-pairs
JC = S // P         # 8 sequence chunks
GROUPS = [[0, 1, 2, 3], [4, 5, 6, 7]]
SCALE = 1.0 / 8.0   # 1/sqrt(DH)
WS = 16.0           # fp8 weight scale

f8 = mybir.dt.float8e4
bf16 = mybir.dt.bfloat16
f32 = mybir.dt.float32
AF = mybir.ActivationFunctionType
ALU = mybir.AluOpType
DR = mybir.MatmulPerfMode.DoubleRow
nf8 = ml_dtypes.float8_e4m3
nbf16 = ml_dtypes.bfloat16

# bf16 weights [H, H] (keys in the kernel's dram map)
WB = ["vq_w", "vk_w", "vv_w", "dv_w", "van_fc_w", "WV_w", "vo0", "do0",
      "diff_fus_w", "van_fus_w", "final_w"]
# fp8 weights (x16) for the DoubleRow gating GEMMs
W8 = ["dth1", "diff_fc_w", "vg0", "vg1", "vo1", "do1", "nf_w"]
BIAS = ["vq_b", "vk_b", "dv_b", "van_fc_b", "diff_fc_b", "diff_out_b",
        "van_out_b", "diff_fus_b", "van_fus_b", "nf_b", "final_b"]


def build(has_vvb: bool):
    nc = bacc.Bacc(None, target_bir_lowering=False, debug=False, num_devices=8)

    xT_d = nc.dram_tensor("xT", [P, KC, RV], bf16, kind="ExternalInput")
    yT_d = nc.dram_tensor("yT", [P, KC, S], bf16, kind="ExternalInput")
    wd = {}
    for w in WB:
        wd[w] = nc.dram_tensor(w, [P, KC, H], bf16, kind="ExternalInput")
    for w in W8:
        wd[w] = nc.dram_tensor(w, [P, KC, H], f8, kind="ExternalInput")
    wd["gate_w"] = nc.dram_tensor("gate_w", [P, 2 * KC], bf16, kind="ExternalInput")
    wd["nf_out_w"] = nc.dram_tensor("nf_out_w", [P, 2 * KC], bf16, kind="ExternalInput")
    bd = {}
    for b in BIAS:
        bd[b] = nc.dram_tensor(b, [P, KC], f32, kind="ExternalInput")
    if has_vvb:
        bd["vv_b"] = nc.dram_tensor("vv_b", [P, KC], f32, kind="ExternalInput")
    out_d = nc.dram_tensor("outT", [H, RV], f32, kind="ExternalOutput")

    with tile.TileContext(nc, num_cores=8) as tc:
        with (
            tc.tile_pool(name="wpool", bufs=6) as wp,
            tc.tile_pool(name="wb16", bufs=7) as wbp,
            tc.tile_pool(name="wsmall", bufs=2) as wsp,
            tc.tile_pool(name="acts", bufs=1) as ap,
            tc.tile_pool(name="loop", bufs=2) as lp,
            tc.tile_pool(name="psum", bufs=8, space="PSUM") as pp,
            tc.tile_pool(name="dram", bufs=6, space="DRAM") as dp,
        ):
            def wload(t, src):
                """Split a weight DMA across both hardware DGE queues."""
                nc.sync.dma_start(t[:, 0:3, :], src[:, 0:3, :])
                nc.scalar.dma_start(t[:, 3:6, :], src[:, 3:6, :])

            def wtile(name):
                t = wp.tile([P, KC, H], f8, name=f"w_{name}", tag="w")
                wload(t, wd[name])
                return t

            def wbtile(name):
                t = wbp.tile([P, KC, H], bf16, name=f"w_{name}", tag="wb")
                wload(t, wd[name])
                return t

            def btile(name):
                t = ap.tile([P, KC], f32, name=f"b_{name}")
                nc.sync.dma_start(t[:], bd[name][:])
                return t

            # ---- warmup collective: pays route/stream setup off-path ------
            zw = ap.tile([P, 1], f32, name="zw")
            nc.vector.memset(zw[:], 0.0)
            ciw = dp.tile([P, 1], f32, name="ciw")
            cow = dp.tile([P, 1], f32, name="cow")
            nc.sync.dma_start(ciw[:], zw[:])
            nc.gpsimd.collective_compute(
                "AllReduce", ALU.add, replica_groups=GROUPS,
                ins=[ciw[:]], outs=[cow[:]])

            ones64 = ap.tile([1, 64], bf16, name="ones64")
            nc.vector.memset(ones64[:], 1.0)
            ones128 = ap.tile([1, P], bf16, name="ones128")
            nc.vector.memset(ones128[:], 1.0)

            # ---------------- q projection (bf16) ---------------------------
            b_vq = btile("vq_b")
            xT = ap.tile([P, KC, RV], bf16, name="xT")
            nc.sync.dma_start(xT[:], xT_d[:])
            w_vq = wbtile("vq_w")

            qT = ap.tile([P, KC, RV], bf16, name="qT")
            for mc in range(KC):
                ps = pp.tile([P, RV], f32, name=f"qps{mc}", tag="sps", bufs=3)
                for kc in range(KC):
                    nc.tensor.matmul(ps[:], w_vq[:, kc, mc * P:(mc + 1) * P],
                                     xT[:, kc, :],
                                     start=(kc == 0), stop=(kc == KC - 1))
                nc.scalar.activation(qT[:, mc, :], ps[:], AF.Identity,
                                     bias=b_vq[:, mc:mc + 1])

            # ---------------- y + K/V weights -------------------------------
            b_vk = btile("vk_b")
            yT = ap.tile([P, KC, S], bf16, name="yT")
            for kc in range(KC):
                eng = nc.sync if kc % 2 == 0 else nc.scalar
                eng.dma_start(yT[:, kc, :], yT_d[:, kc, :])
            w_vk = wbtile("vk_w")
            w_vv = wbtile("vv_w")

            kT = ap.tile([P, KC, S], bf16, name="kT")

            def kproj(mc):
                for nh in range(2):
                    ps = pp.tile([P, 512], f32, name=f"kps{mc}_{nh}",
                                 tag="big", bufs=3)
                    for kc in range(KC):
                        nc.tensor.matmul(
                            ps[:], w_vk[:, kc, mc * P:(mc + 1) * P],
                            yT[:, kc, nh * 512:(nh + 1) * 512],
                            start=(kc == 0), stop=(kc == KC - 1))
                    nc.scalar.activation(kT[:, mc, nh * 512:(nh + 1) * 512],
                                         ps[:], AF.Identity,
                                         bias=b_vk[:, mc:mc + 1])

            v_aug = ap.tile([P, JC, NH, DH + 1], bf16, name="v_aug")
            nc.vector.memset(v_aug[:, :, :, DH:DH + 1], 1.0)

            def vproj(jc, cg):
                ps = pp.tile([P, 384], f32, name=f"vps{jc}_{cg}",
                             tag="big", bufs=3)
                for kc in range(KC):
                    nc.tensor.matmul(
                        ps[:], yT[:, kc, jc * P:(jc + 1) * P],
                        w_vv[:, kc, cg * 384:(cg + 1) * 384],
                        start=(kc == 0), stop=(kc == KC - 1))
                nc.scalar.activation(
                    v_aug[:, jc, cg * 6:(cg + 1) * 6, 0:DH],
                    ps[:].rearrange("p (h d) -> p h d", d=DH), AF.Copy)

            # ---------------- diff-branch constants (m, bias2) --------------
            b_dv = btile("dv_b")
            w_dv = wbtile("dv_w")
            b_dout = btile("diff_out_b")
            w_do0 = wbtile("do0")

            ybt = ap.tile([P, KC], bf16, name="ybt")
            yb32 = ap.tile([P, KC], f32, name="yb32")
            for kc in range(KC):
                nc.vector.tensor_reduce(yb32[:, kc:kc + 1], yT[:, kc, :],
                                        axis=mybir.AxisListType.X, op=ALU.add)
            nc.vector.tensor_scalar_mul(ybt[:], yb32[:], 1.0 / S)

            def vchain(wt, rh, bias_t, name):
                out = ap.tile([P, KC], f32, name=name)
                for mc in range(KC):
                    ps = pp.tile([P, 1], f32, name=f"{name}ps{mc}",
                                 tag="sps", bufs=3)
                    for kc in range(KC):
                        nc.tensor.matmul(ps[:], wt[:, kc, mc * P:(mc + 1) * P],
                                         rh[:, kc:kc + 1],
                                         start=(kc == 0), stop=(kc == KC - 1))
                    nc.scalar.activation(out[:, mc:mc + 1], ps[:], AF.Identity,
                                         bias=bias_t[:, mc:mc + 1])
                return out

            m32 = vchain(w_dv, ybt, b_dv, "m32")
            mbf = ap.tile([P, KC], bf16, name="mbf")
            nc.vector.tensor_copy(mbf[:], m32[:])
            bias2 = vchain(w_do0, mbf, b_dout, "bias2")

            if has_vvb:
                b_vv = btile("vv_b")

            # ---------------- attention, pipelined with K projection --------
            vanT = ap.tile([P, KC, RV], bf16, name="vanT")

            def head_tail(h, pv):
                hc, ho = h // 2, (h % 2) * 64
                iz = lp.tile([1, RV], bf16, name=f"iz{h}", tag="iz", bufs=2)
                with nc.allow_low_precision(reason="invZ bf16, modeled"):
                    nc.vector.reciprocal(iz[:], pv[DH:DH + 1, :])
                bc = pp.tile([64, RV], f32, name=f"bc{h}", tag="sps", bufs=3)
                nc.tensor.matmul(bc[:], ones64[:], iz[:],
                                 start=True, stop=True)
                bcs = lp.tile([64, RV], bf16, name=f"bcs{h}", tag="bcs", bufs=2)
                nc.vector.tensor_copy(bcs[:], bc[:])
                nc.vector.tensor_mul(vanT[ho:ho + 64, hc, :], pv[0:DH, :],
                                     bcs[:])
                if has_vvb:
                    nc.vector.tensor_scalar_add(vanT[ho:ho + 64, hc, :],
                                                vanT[ho:ho + 64, hc, :],
                                                b_vv[ho:ho + 64, hc:hc + 1])

            def attend(hp):
                h0, h1 = 2 * hp, 2 * hp + 1
                hc = hp
                e0 = lp.tile([P, JC, RV], bf16, name=f"expT{h0}",
                             tag="expT", bufs=4)
                e1_ = lp.tile([P, JC, RV], bf16, name=f"expT{h1}",
                              tag="expT", bufs=4)
                for jp in range(JC // 2):
                    for (h, ex) in ((h0, e0), (h1, e1_)):
                        ho = (h % 2) * 64
                        sps = pp.tile([P, 512], f32, name=f"s{h}_{jp}",
                                      tag="big", bufs=3)
                        for half in range(2):
                            jc = 2 * jp + half
                            nc.tensor.matmul(
                                sps[:, half * RV:(half + 1) * RV],
                                kT[ho:ho + 64, hc, jc * P:(jc + 1) * P],
                                qT[ho:ho + 64, hc, :],
                                start=True, stop=True)
                        nc.scalar.activation(
                            ex[:, 2 * jp:2 * jp + 2, :],
                            sps[:].rearrange("p (a b) -> p a b", b=RV),
                            AF.Exp, scale=SCALE)
                pv0 = pp.tile([DH + 1, RV], f32, name=f"pv{h0}", tag="pv", bufs=2)
                pv1 = pp.tile([DH + 1, RV], f32, name=f"pv{h1}", tag="pv", bufs=2)
                for jc in range(JC):
                    nc.tensor.matmul(pv0[:], v_aug[:, jc, h0, :], e0[:, jc, :],
                                     start=(jc == 0), stop=(jc == JC - 1))
                    nc.tensor.matmul(pv1[:], v_aug[:, jc, h1, :], e1_[:, jc, :],
                                     start=(jc == 0), stop=(jc == JC - 1))
                head_tail(h0, pv0)
                head_tail(h1, pv1)

            # interleave K-projection chunks with the head pairs they feed
            kproj(0)
            for jc in range(JC):
                vproj(jc, 0)
            attend(0)
            kproj(1)
            attend(1)
            kproj(2)
            attend(2)
            for jc in range(JC):
                vproj(jc, 1)
            kproj(3)
            attend(3)
            kproj(4)
            attend(4)
            kproj(5)
            attend(5)

            # ---------------- gating network --------------------------------
            def gemm16(wt, func, at, name, bias_t=None, out_dt=bf16,
                       accum_t=None):
                """bf16 GEMM: out = func(W^T at + bias)."""
                out = ap.tile([P, KC, RV], out_dt, name=name)
                for mc in range(KC):
                    ps = pp.tile([P, RV], f32, name=f"{name}ps{mc}",
                                 tag="sps", bufs=3)
                    for kc in range(KC):
                        nc.tensor.matmul(ps[:], wt[:, kc, mc * P:(mc + 1) * P],
                                         at[:, kc, :],
                                         start=(kc == 0), stop=(kc == KC - 1))
                    nc.scalar.activation(
                        out[:, mc, :], ps[:], func,
                        bias=(bias_t[:, mc:mc + 1] if bias_t is not None
                              else 0.0),
                        accum_out=(accum_t[:, mc:mc + 1]
                                   if accum_t is not None else None))
                return out

            def gemm8(wt, func, at, name, sA, bias_t=None, out_dt=bf16,
                      accum_t=None, pre=None, pre_s=1.0):
                """fp8 DoubleRow GEMM: out = func((W8^T at)*sA + bias)."""
                out = ap.tile([P, KC, RV], out_dt, name=name)
                for mc in range(KC):
                    ms = slice(mc * P, (mc + 1) * P)
                    ps = pp.tile([P, RV], f32, name=f"{name}ps{mc}",
                                 tag="sps", bufs=3)
                    for c in range(KP):
                        cs = slice(2 * c, 2 * c + 2)
                        nc.tensor.matmul(ps[:], wt[:, cs, ms], at[:, cs, :],
                                         start=(c == 0), stop=(c == KP - 1),
                                         perf_mode=DR)
                    if pre is not None:
                        nc.vector.scalar_tensor_tensor(
                            ps[:], ps[:], pre_s, pre[:, mc, :],
                            op0=ALU.mult, op1=ALU.add)
                        sAx = 1.0
                    else:
                        sAx = sA
                    nc.scalar.activation(
                        out[:, mc, :], ps[:], func, scale=sAx,
                        bias=(bias_t[:, mc:mc + 1] if bias_t is not None
                              else 0.0),
                        accum_out=(accum_t[:, mc:mc + 1]
                                   if accum_t is not None else None))
                return out

            def allreduce6(part, name):
                ci = dp.tile([P, KC], f32, name=f"ci_{name}")
                co = dp.tile([P, KC], f32, name=f"co_{name}")
                nc.sync.dma_start(ci[:], part[:])
                nc.gpsimd.collective_compute(
                    "AllReduce", ALU.add, replica_groups=GROUPS,
                    ins=[ci[:]], outs=[co[:]])
                z = ap.tile([P, KC], f32, name=f"z_{name}")
                nc.sync.dma_start(z[:], co[:])
                return z

            # theta2 = tanh(van @ van_fc + b) -> fp8
            w_vfc = wbtile("van_fc_w")
            b_vfc = btile("van_fc_b")
            t2_8 = gemm16(w_vfc, AF.Tanh, vanT, "t2_8", bias_t=b_vfc,
                          out_dt=f8)

            # e1 = exp(theta2 @ d_theta_w[H:] / WS);  z1 via AllReduce
            w_dth1 = wtile("dth1")
            part1 = ap.tile([P, KC], f32, name="part1")
            e1t = gemm8(w_dth1, AF.Exp, t2_8, "e1t", 1.0 / WS, accum_t=part1)
            z1 = allreduce6(part1, "z1")

            # --- AllReduce-1 bubble fillers (independent of z1) -------------
            w_WV = wbtile("WV_w")
            g1_8 = gemm16(w_WV, AF.Tanh, vanT, "g1_8", out_dt=f8)
            w_vg0 = wtile("vg0")
            z2a = gemm8(w_vg0, AF.Identity, g1_8, "z2a", 1.0 / WS)
            w_vo0 = wbtile("vo0")
            b_vo = btile("van_out_b")
            voa = gemm16(w_vo0, AF.Identity, vanT, "voa", bias_t=b_vo,
                         out_dt=f32)

            # dth8 = 16384 * m * e1 / z1
            s1 = ap.tile([P, KC], f32, name="s1")
            nc.vector.reciprocal(s1[:], z1[:])
            nc.vector.scalar_tensor_tensor(s1[:], s1[:], 16384.0, m32[:],
                                           op0=ALU.mult, op1=ALU.mult)
            dth8 = ap.tile([P, KC, RV], f8, name="dth8")
            for mc in range(KC):
                nc.vector.tensor_scalar_mul(dth8[:, mc, :], e1t[:, mc, :],
                                            s1[:, mc:mc + 1])

            w_dfc = wtile("diff_fc_w")
            b_dfc = btile("diff_fc_b")
            g2_8 = gemm8(w_dfc, AF.Tanh, dth8, "g2_8", 1.0 / (WS * 16384.0),
                         bias_t=b_dfc, out_dt=f8)

            w_vg1 = wtile("vg1")
            part2 = ap.tile([P, KC], f32, name="part2")
            e2t = gemm8(w_vg1, AF.Exp, g2_8, "e2t", 1.0, accum_t=part2,
                        pre=z2a, pre_s=1.0 / WS)
            z2 = allreduce6(part2, "z2")

            # --- AllReduce-2 bubble fillers (diff tail) ---------------------
            w_do1 = wtile("do1")
            dout = gemm8(w_do1, AF.Tanh, dth8, "dout", 1.0 / (WS * 16384.0),
                         bias_t=bias2)
            w_dfus = wbtile("diff_fus_w")
            b_dfus = btile("diff_fus_b")
            dfus = gemm16(w_dfus, AF.Tanh, dout, "dfus", bias_t=b_dfus)

            wg = wsp.tile([P, 2 * KC], bf16, name="ws_gate", tag="ws")
            nc.sync.dma_start(wg[:], wd["gate_w"][:])
            wn = wsp.tile([P, 2 * KC], bf16, name="ws_nf", tag="ws")
            nc.sync.dma_start(wn[:], wd["nf_out_w"][:])
            w_vfus = wbtile("van_fus_w")
            b_vfus = btile("van_fus_b")
            w_fin = wbtile("final_w")
            b_fin = btile("final_b")
            b_nf = btile("nf_b")
            w_nf = wtile("nf_w")
            w_vo1 = wtile("vo1")

            # ag8 = 2048 * van * e2 / z2
            s2 = ap.tile([P, KC], f32, name="s2")
            nc.vector.reciprocal(s2[:], z2[:])
            nc.vector.tensor_scalar_mul(s2[:], s2[:], 2048.0)
            ag8 = ap.tile([P, KC, RV], f8, name="ag8")
            for mc in range(KC):
                nc.vector.scalar_tensor_tensor(
                    ag8[:, mc, :], e2t[:, mc, :], s2[:, mc:mc + 1],
                    vanT[:, mc, :], op0=ALU.mult, op1=ALU.mult)

            vout = gemm8(w_vo1, AF.Tanh, ag8, "vout", 1.0,
                         pre=voa, pre_s=1.0 / (WS * 2048.0))
            vfus = gemm16(w_vfus, AF.Tanh, vout, "vfus", bias_t=b_vfus)

            # gate = sigmoid([dfus, vfus] @ gate_w) via exp + reciprocal
            def vec_sigmoid(wt, a0, a1, name):
                ps = pp.tile([1, RV], f32, name=f"{name}ps", tag="sps", bufs=3)
                i = 0
                for at, base in ((a0, 0), (a1, KC)):
                    for kc in range(KC):
                        nc.tensor.matmul(ps[:], wt[:, base + kc:base + kc + 1],
                                         at[:, kc, :],
                                         start=(i == 0), stop=(i == 2 * KC - 1))
                        i += 1
                out = ap.tile([1, RV], bf16, name=f"v_{name}")
                nc.scalar.activation(out[:], ps[:], AF.Exp, scale=-1.0)
                nc.vector.tensor_scalar_add(out[:], out[:], 1.0)
                with nc.allow_low_precision(reason="sigmoid bf16, modeled"):
                    nc.vector.reciprocal(out[:], out[:])
                return out

            g = vec_sigmoid(wg, dfus, vfus, "gate")
            gbc = pp.tile([P, RV], f32, name="gbc", tag="sps", bufs=3)
            nc.tensor.matmul(gbc[:], ones128[:], g[:], start=True, stop=True)
            gbcs = ap.tile([P, RV], bf16, name="gbcs")
            nc.vector.tensor_copy(gbcs[:], gbc[:])

            fus = ap.tile([P, KC, RV], bf16, name="fus")
            fus8 = ap.tile([P, KC, RV], f8, name="fus8")
            for mc in range(KC):
                t1 = lp.tile([P, RV], bf16, name=f"ft1_{mc}", tag="ft1", bufs=4)
                nc.vector.tensor_sub(t1[:], vfus[:, mc, :], dfus[:, mc, :])
                nc.vector.tensor_mul(t1[:], t1[:], gbcs[:])
                nc.vector.tensor_add(fus[:, mc, :], t1[:], dfus[:, mc, :])
                nc.vector.tensor_copy(fus8[:, mc, :], fus[:, mc, :])

            tnf = gemm8(w_nf, AF.Identity, fus8, "tnf", 1.0 / WS, bias_t=b_nf)
            nfv = vec_sigmoid(wn, vanT, tnf, "nf")
            nbc = pp.tile([P, RV], f32, name="nbc", tag="sps", bufs=3)
            nc.tensor.matmul(nbc[:], ones128[:], nfv[:], start=True, stop=True)
            nbcs = ap.tile([P, RV], f32, name="nbcs")
            nc.vector.tensor_copy(nbcs[:], nbc[:])

            # out = nf * tanh(fus @ final_w + b), streamed out per chunk
            outT = ap.tile([P, KC, RV], f32, name="outT")
            od = out_d.rearrange("(mc p) n -> mc p n", p=P)
            for mc in range(KC):
                ps = pp.tile([P, RV], f32, name=f"fps{mc}", tag="sps", bufs=3)
                for kc in range(KC):
                    nc.tensor.matmul(ps[:], w_fin[:, kc, mc * P:(mc + 1) * P],
                                     fus[:, kc, :],
                                     start=(kc == 0), stop=(kc == KC - 1))
                ft = lp.tile([P, RV], bf16, name=f"ftn{mc}", tag="ftn", bufs=2)
                nc.scalar.activation(ft[:], ps[:], AF.Tanh,
                                     bias=b_fin[:, mc:mc + 1])
                nc.vector.tensor_mul(outT[:, mc, :], ft[:], nbcs[:])
                nc.sync.dma_start(od[mc], outT[:, mc, :])

    nc.compile()
    return nc


_CACHE = {}


def _wlay(w, dt):
    """[H, M] -> [P, KC, M] partition-major (lhsT tile layout)."""
    w = np.asarray(w, np.float32).astype(dt)
    return np.ascontiguousarray(
        w.reshape(KC, P, w.shape[1]).transpose(1, 0, 2))


def _blay(b):
    return np.ascontiguousarray(np.asarray(b, np.float32).reshape(KC, P).T)


def prep_inputs(inputs):
    """Host-side layout + quantization. Returns (has_vvb, per-core maps)."""
    x = np.asarray(inputs["x"], np.float32)
    y = np.asarray(inputs["y"], np.float32)
    g = {k: np.asarray(v, np.float32) for k, v in inputs.items()}

    base = {}
    bsrc = {"vq_w": g["vq_w"], "vk_w": g["vk_w"], "vv_w": g["vv_w"],
            "dv_w": g["dv_w"], "van_fc_w": g["van_fc_w"], "WV_w": g["WV_w"],
            "vo0": g["van_out_w"][:H], "do0": g["diff_out_w"][:H],
            "diff_fus_w": g["diff_fus_w"], "van_fus_w": g["van_fus_w"],
            "final_w": g["final_w"]}
    for n, w in bsrc.items():
        base[n] = _wlay(w, nbf16)
    fsrc = {"dth1": g["d_theta_w"][H:], "diff_fc_w": g["diff_fc_w"],
            "vg0": g["v_gamma_w"][:H], "vg1": g["v_gamma_w"][H:],
            "vo1": g["van_out_w"][H:], "do1": g["diff_out_w"][H:],
            "nf_w": g["nf_w"]}
    for n, w in fsrc.items():
        base[n] = np.ascontiguousarray(
            (w * WS).astype(nf8).reshape(KC, P, H).transpose(1, 0, 2))
    for n in ("gate_w", "nf_out_w"):
        base[n] = np.ascontiguousarray(
            g[n].reshape(2 * KC, P).T.astype(nbf16))
    for b in BIAS:
        base[b] = _blay(g[b])
    has_vvb = bool(np.any(g["vv_b"] != 0))
    if has_vvb:
        base["vv_b"] = _blay(g["vv_b"])

    yts = []
    for b in range(B):
        yt = np.ascontiguousarray(y[b].T).astype(nbf16)       # [H, S]
        yts.append(np.ascontiguousarray(
            yt.reshape(KC, P, S).transpose(1, 0, 2)))
    xt = np.ascontiguousarray(x.reshape(B * S, H).T).astype(nbf16)
    xt = xt.reshape(KC, P, B * S).transpose(1, 0, 2)

    in_maps = []
    for c in range(8):
        mm = dict(base)
        mm["xT"] = np.ascontiguousarray(xt[:, :, c * RV:(c + 1) * RV])
        mm["yT"] = yts[c // 4]
        in_maps.append(mm)
    return has_vvb, in_maps


def kernel(**inputs):
    has_vvb, in_maps = prep_inputs(inputs)
    if has_vvb not in _CACHE:
        _CACHE[has_vvb] = build(has_vvb)
    nc = _CACHE[has_vvb]
    res = run_bass_kernel_spmd(nc, in_maps, core_ids=list(range(8)))
    full = np.concatenate([res.results[c]["outT"] for c in range(8)], axis=1)
    return np.ascontiguousarray(full.T.reshape(B, S, H)).astype(np.float32)


if __name__ == "__main__":
    rng = np.random.default_rng(0)
    ins = {"x": rng.standard_normal((B, S, H)).astype(np.float32),
           "y": rng.standard_normal((B, S, H)).astype(np.float32)}
    for w in ["vq_w", "vk_w", "vv_w", "dq_w", "dk_w", "dv_w", "van_fc_w",
              "WD_w", "WV_w", "diff_fc_w", "diff_fus_w", "van_fus_w",
              "nf_w", "final_w"]:
        ins[w] = (rng.standard_normal((H, H)) * 0.02).astype(np.float32)
    for w in ["d_theta_w", "v_gamma_w", "diff_out_w", "van_out_w"]:
        ins[w] = (rng.standard_normal((2 * H, H)) * 0.02).astype(np.float32)
    ins["gate_w"] = (rng.standard_normal((2 * H, 1)) * 0.02).astype(np.float32)
    ins["nf_out_w"] = (rng.standard_normal((2 * H, 1)) * 0.02).astype(np.float32)
    for b in ["vq_b", "vk_b", "vv_b", "dq_b", "dk_b", "dv_b", "van_fc_b",
              "d_theta_b", "diff_fc_b", "v_gamma_b", "diff_out_b",
              "van_out_b", "diff_fus_b", "van_fus_b", "nf_b", "final_b"]:
        ins[b] = np.zeros(H, np.float32)
    out = kernel(**ins)
    print("out", out.shape, out.dtype, np.abs(out).mean())
